# revision 20
# baseline (speedup 1.0000x reference)
"""Tacotron2 location-sensitive attention on 8 TRN2 NeuronCores.

Data-parallel over batch B=128 -> 16 rows per core; params replicated.
Fully per-batch-row pipelined (no softmax barrier):
  pq    = hidden @ query_W.T                           (PE, f32, once)
  loc   = conv1d(aw_cat) @ loc_W.T  -- folded CW2 = loc_W @ conv_W, conv as
          one matmul over im2col'd input (host-marshalled)
  th    = tanh(pq + loc + pm)                          (PE + DVE + ACT)
  eT    = th.T @ v  as 16 N=1 matmuls -> (128, 16) transposed energies,
          which IS the stationary-weight layout phase 2 needs
  wu    = exp(eT + mask)      (no max-shift: |e| <= ||v||_1 ~ 10, f32-safe)
  ctx   = (wu @ memory) / sum(wu)                      (PE matvec + scalar norm)

Matmul-facing tensors are bf16 (PE fp32 streams at 1/4 rate); psum
accumulation, softmax arithmetic and outputs stay f32.  memory is host-cast
to bf16 and pre-tiled to (2, 128, 4096) contiguous blocks per batch row.
"""

import os
import numpy as np
from contextlib import ExitStack

import concourse.bass as bass
import concourse.bacc as bacc
import concourse.tile as tile
from concourse import bass_isa, mybir, masks
from concourse.bass_utils import run_bass_kernel_spmd

F32 = mybir.dt.float32
BF16 = mybir.dt.bfloat16
ts = bass.ts

N_CORES = 8
B, T = 128, 2048
BP = B // N_CORES          # 16 batch rows per core
RNN, ATT, ENC = 1024, 128, 512
NF, KS, PAD = 32, 31, 15
CK = 2 * KS                # 62 im2col rows
NT = T // 128              # 16 T-slices of 128

_TRACE = os.environ.get("BASS_KERNEL_TRACE", "0") == "1"
LAST_RESULT = None
_NC_CACHE = None


def _build_nc():
    nc = bacc.Bacc("TRN2", target_bir_lowering=False, debug=False,
                   num_devices=N_CORES)

    hiddenT = nc.dram_tensor("hiddenT", [RNN, BP], F32, kind="ExternalInput").ap()
    qWT = nc.dram_tensor("qWT", [RNN, ATT], F32, kind="ExternalInput").ap()
    xs = nc.dram_tensor("xs", [BP, CK, T], BF16, kind="ExternalInput").ap()
    cwr = nc.dram_tensor("cwr", [NF, CK], F32, kind="ExternalInput").ap()
    locWT = nc.dram_tensor("locWT", [NF, ATT], F32, kind="ExternalInput").ap()
    vT = nc.dram_tensor("vT", [ATT, 1], BF16, kind="ExternalInput").ap()
    pmT = nc.dram_tensor("pmT", [BP, ATT, T], BF16, kind="ExternalInput").ap()
    # mask in transposed-chunk layout: [b, p, c] = maskadd[b, c*128 + p]
    maskT = nc.dram_tensor("maskT", [BP, 128, NT], F32, kind="ExternalInput").ap()
    # memory pre-tiled: [b, h, p, c*512+d] = mem[b, h*1024 + c*128 + p, d]
    mem = nc.dram_tensor("mem", [BP, 2, 128, 8 * ENC], BF16,
                         kind="ExternalInput").ap()

    out_ctx = nc.dram_tensor("out_ctx", [BP, ENC], F32, kind="ExternalOutput").ap()
    out_w = nc.dram_tensor("out_w", [BP, T], F32, kind="ExternalOutput").ap()

    with tile.TileContext(nc) as tc, ExitStack() as ctx:
        const_pool = ctx.enter_context(tc.tile_pool(name="const", bufs=1))
        xs_pool = ctx.enter_context(tc.tile_pool(name="xs", bufs=4))
        pm_pool = ctx.enter_context(tc.tile_pool(name="pm", bufs=4))
        mk_pool = ctx.enter_context(tc.tile_pool(name="mk", bufs=3))
        s_pool = ctx.enter_context(tc.tile_pool(name="s", bufs=4))
        th_pool = ctx.enter_context(tc.tile_pool(name="th", bufs=4))
        wu_pool = ctx.enter_context(tc.tile_pool(name="wu", bufs=3))
        sc_pool = ctx.enter_context(tc.tile_pool(name="sc", bufs=4))
        ow_pool = ctx.enter_context(tc.tile_pool(name="ow", bufs=3))
        mem_pool = ctx.enter_context(tc.tile_pool(name="mem", bufs=12))
        o_pool = ctx.enter_context(tc.tile_pool(name="o", bufs=4))
        ps_loc_pool = ctx.enter_context(tc.tile_pool(name="psloc", bufs=3, space="PSUM"))
        ps_e_pool = ctx.enter_context(tc.tile_pool(name="pse", bufs=2, space="PSUM"))
        ps_mc_pool = ctx.enter_context(tc.tile_pool(name="psmc", bufs=2, space="PSUM"))
        ps_ow_pool = ctx.enter_context(tc.tile_pool(name="psow", bufs=1, space="PSUM"))

        # ---- constants into SBUF ----
        qwt_t = const_pool.tile([128, (RNN // 128) * ATT], F32)
        ht_t = const_pool.tile([128, (RNN // 128) * BP], F32)
        for c in range(RNN // 128):
            nc.sync.dma_start(qwt_t[:, ts(c, ATT)], qWT[c * 128:(c + 1) * 128, :])
            nc.sync.dma_start(ht_t[:, ts(c, BP)], hiddenT[c * 128:(c + 1) * 128, :])
        cwr_t = const_pool.tile([NF, CK], F32)
        nc.sync.dma_start(cwr_t[:], cwr)
        locwt_t = const_pool.tile([NF, ATT], F32)
        nc.sync.dma_start(locwt_t[:], locWT)
        vt_t = const_pool.tile([ATT, 1], BF16)
        nc.sync.dma_start(vt_t[:], vT)
        ident_f = const_pool.tile([128, 128], F32)
        masks.make_identity(nc, ident_f[:])

        # ---- pq = query_W @ hidden.T : (ATT=128, BP=16), f32 ----
        ps_pq = ps_mc_pool.tile([ATT, BP], F32, tag="mc")
        for c in range(RNN // 128):
            nc.tensor.matmul(ps_pq[:], qwt_t[:, ts(c, ATT)], ht_t[:, ts(c, BP)],
                             start=(c == 0), stop=(c == RNN // 128 - 1))
        pq_t = const_pool.tile([ATT, BP], F32)
        nc.vector.tensor_copy(pq_t[:], ps_pq[:])

        # ---- CW2T[(c,k), a] = sum_f conv_W[f,(c,k)] loc_W[a,f] : (62, 128) ----
        ps_cw = ps_mc_pool.tile([CK, ATT], F32, tag="mc")
        nc.tensor.matmul(ps_cw[:], cwr_t[:], locwt_t[:], start=True, stop=True)
        cw2t_t = const_pool.tile([CK, ATT], BF16)
        nc.vector.tensor_copy(cw2t_t[:], ps_cw[:])

        mem_engs = [nc.sync, nc.gpsimd]
        for b in range(BP):
            # ---- energies (transposed): ps_eT[p, c] = e[b, c*128+p] ----
            xs_t = xs_pool.tile([CK, T], BF16)
            nc.gpsimd.dma_start(xs_t[:], xs[b])
            pm_t = pm_pool.tile([ATT, T], BF16)
            nc.gpsimd.dma_start(pm_t[:], pmT[b])
            mk_t = mk_pool.tile([128, NT], F32)
            nc.sync.dma_start(mk_t[:], maskT[b])

            ps_eT = ps_e_pool.tile([128, NT], F32)
            for c in range(4):
                ps_loc = ps_loc_pool.tile([ATT, 512], F32)
                nc.tensor.matmul(ps_loc[:], cw2t_t[:], xs_t[:, ts(c, 512)],
                                 start=True, stop=True)
                s_t = s_pool.tile([ATT, 512], F32)
                nc.vector.tensor_add(s_t[:], ps_loc[:], pm_t[:, ts(c, 512)])
                th_t = th_pool.tile([ATT, 512], BF16)
                nc.scalar.activation(th_t[:], s_t[:],
                                     mybir.ActivationFunctionType.Tanh,
                                     bias=pq_t[:, b:b + 1])
                for sl in range(4):
                    si = c * 4 + sl
                    nc.tensor.matmul(ps_eT[:, si:si + 1],
                                     th_t[:, ts(sl, 128)], vt_t[:],
                                     start=True, stop=True)

            # ---- masked exp (no max shift; |e| <= ||v||_1), sums ----
            emT_t = wu_pool.tile([128, NT], F32, tag="em")
            nc.vector.tensor_add(emT_t[:], ps_eT[:], mk_t[:])
            wuf_t = wu_pool.tile([128, NT], F32, tag="wuf")
            acc_t = sc_pool.tile([128, 1], F32, tag="acc")
            nc.scalar.activation(wuf_t[:], emT_t[:],
                                 mybir.ActivationFunctionType.Exp,
                                 accum_out=acc_t[:])
            wub_t = wu_pool.tile([128, NT], BF16, tag="wub")
            nc.vector.tensor_copy(wub_t[:], wuf_t[:])
            sumb_t = sc_pool.tile([128, 1], F32, tag="sumb")
            nc.gpsimd.partition_all_reduce(sumb_t[:], acc_t[:], channels=128,
                                           reduce_op=bass_isa.ReduceOp.add)
            rinv128_t = sc_pool.tile([128, 1], F32, tag="r128")
            nc.vector.reciprocal(rinv128_t[:], sumb_t[:])

            # ---- normalized weights row -> out_w[b] ----
            wnT_t = wu_pool.tile([128, NT], F32, tag="wn")
            nc.vector.tensor_scalar_mul(wnT_t[:], wuf_t[:], rinv128_t[:])
            ps_ow = ps_ow_pool.tile([NT, 128], F32)
            nc.tensor.transpose(ps_ow[:], wnT_t[:], ident_f[:])
            ow_t = ow_pool.tile([NT, 128], F32)
            nc.vector.tensor_copy(ow_t[:], ps_ow[:])
            nc.sync.dma_start(out_w[b:b + 1, :], ow_t[:])

            # ---- context: ctx[b] = (sum_t wu[t] mem[t, :]) * rinv ----
            ps_c = ps_mc_pool.tile([1, ENC], F32, tag="mc")
            for h in range(2):
                mem_t = mem_pool.tile([128, 8 * ENC], BF16)
                mem_engs[(2 * b + h) % 2].dma_start(mem_t[:], mem[b, h])
                for c in range(8):
                    si = h * 8 + c
                    nc.tensor.matmul(ps_c[:], wub_t[:, si:si + 1],
                                     mem_t[:, ts(c, ENC)],
                                     start=(si == 0), stop=(si == NT - 1))
            ctx_t = o_pool.tile([1, ENC], F32)
            nc.vector.tensor_scalar_mul(ctx_t[:], ps_c[:], rinv128_t[0:1, :])
            nc.scalar.dma_start(out_ctx[b:b + 1, :], ctx_t[:])

    nc.compile()
    return nc


def _marshal(inputs):
    """Full inputs -> per-core in_maps (host-side layout/dtype only)."""
    from ml_dtypes import bfloat16

    hid = np.ascontiguousarray(np.asarray(inputs["attention_hidden_state"], np.float32))
    memory = np.asarray(inputs["memory"], np.float32)
    pm = np.asarray(inputs["processed_memory"], np.float32)
    awc = np.asarray(inputs["attention_weights_cat"], np.float32)
    mask = np.asarray(inputs["mask"])
    qW = np.asarray(inputs["query_W"], np.float32)
    cW = np.asarray(inputs["conv_W"], np.float32)
    lW = np.asarray(inputs["loc_W"], np.float32)
    vW = np.asarray(inputs["v_W"], np.float32)

    hT = np.ascontiguousarray(hid.T)                       # (RNN, B)
    qWT = np.ascontiguousarray(qW.T)                       # (RNN, ATT)
    cwr = np.ascontiguousarray(cW.reshape(NF, CK))         # (32, 62)
    locWT = np.ascontiguousarray(lW.T)                     # (32, 128)
    vT = np.ascontiguousarray(vW.T).astype(bfloat16)       # (128, 1)

    xp = np.zeros((B, 2, T + 2 * PAD), np.float32)
    xp[:, :, PAD:PAD + T] = awc
    s0, s1, s2 = xp.strides
    xs_view = np.lib.stride_tricks.as_strided(
        xp, shape=(B, 2, KS, T), strides=(s0, s1, s2, s2))
    xs = np.ascontiguousarray(xs_view.reshape(B, CK, T)).astype(bfloat16)

    pmT = np.ascontiguousarray(pm.transpose(0, 2, 1)).astype(bfloat16)
    maskadd = np.where(mask, np.float32(-1e30), np.float32(0.0)).astype(np.float32)
    maskT = np.ascontiguousarray(
        maskadd.reshape(B, NT, 128).transpose(0, 2, 1))    # (B, 128, NT)
    # pre-tile memory: [b, h, p, c*512+d] = mem[b, h*1024 + c*128 + p, d]
    mem_bf = memory.astype(bfloat16)
    mem_tiled = np.ascontiguousarray(
        mem_bf.reshape(B, 2, 8, 128, ENC).transpose(0, 1, 3, 2, 4)
        .reshape(B, 2, 128, 8 * ENC))

    in_maps = []
    for c in range(N_CORES):
        sl = slice(c * BP, (c + 1) * BP)
        in_maps.append({
            "hiddenT": np.ascontiguousarray(hT[:, sl]),
            "qWT": qWT,
            "xs": xs[sl],
            "cwr": cwr,
            "locWT": locWT,
            "vT": vT,
            "pmT": pmT[sl],
            "maskT": maskT[sl],
            "mem": mem_tiled[sl],
        })
    return in_maps


def kernel(**inputs):
    global _NC_CACHE, LAST_RESULT
    if _NC_CACHE is None:
        _NC_CACHE = _build_nc()
    nc = _NC_CACHE
    in_maps = _marshal(inputs)
    res = run_bass_kernel_spmd(nc, in_maps, core_ids=list(range(N_CORES)),
                               trace=_TRACE)
    LAST_RESULT = res
    ctx = np.concatenate([r["out_ctx"] for r in res.results], axis=0)
    w = np.concatenate([r["out_w"] for r in res.results], axis=0)
    return ctx, w


# revision 21
# speedup vs baseline: 1.0761x; 1.0761x over previous
"""Tacotron2 location-sensitive attention on 8 TRN2 NeuronCores.

Data-parallel over batch B=128 -> 16 rows per core; params replicated.
Fully per-batch-row pipelined (no softmax barrier):
  pq    = hidden @ query_W.T                           (PE, f32, once)
  loc   = conv1d(aw_cat) @ loc_W.T  -- folded CW2 = loc_W @ conv_W, conv as
          one matmul over im2col'd input (host-marshalled)
  th    = tanh(pq + loc + pm)                          (PE + DVE + ACT)
  eT    = th.T @ v  as 16 N=1 matmuls -> (128, 16) transposed energies,
          which IS the stationary-weight layout phase 2 needs
  wu    = exp(eT + mask)      (no max-shift: |e| <= ||v||_1 ~ 10, f32-safe)
  ctx   = (wu @ memory) / sum(wu)                      (PE matvec + scalar norm)

Matmul-facing tensors are bf16 (PE fp32 streams at 1/4 rate); psum
accumulation, softmax arithmetic and outputs stay f32.  memory is host-cast
to bf16 and pre-tiled to (2, 128, 4096) contiguous blocks per batch row.
"""

import os
import numpy as np
from contextlib import ExitStack

import concourse.bass as bass
import concourse.bacc as bacc
import concourse.tile as tile
from concourse import bass_isa, mybir, masks
from concourse.bass_utils import run_bass_kernel_spmd

F32 = mybir.dt.float32
BF16 = mybir.dt.bfloat16
ts = bass.ts

N_CORES = 8
B, T = 128, 2048
BP = B // N_CORES          # 16 batch rows per core
RNN, ATT, ENC = 1024, 128, 512
NF, KS, PAD = 32, 31, 15
CK = 2 * KS                # 62 im2col rows
NT = T // 128              # 16 T-slices of 128

_TRACE = os.environ.get("BASS_KERNEL_TRACE", "0") == "1"
LAST_RESULT = None
_NC_CACHE = None


def _build_nc():
    nc = bacc.Bacc("TRN2", target_bir_lowering=False, debug=False,
                   num_devices=N_CORES)

    hiddenT = nc.dram_tensor("hiddenT", [RNN, BP], F32, kind="ExternalInput").ap()
    qWT = nc.dram_tensor("qWT", [RNN, ATT], F32, kind="ExternalInput").ap()
    xs = nc.dram_tensor("xs", [BP, CK, T], BF16, kind="ExternalInput").ap()
    cwr = nc.dram_tensor("cwr", [NF, CK], F32, kind="ExternalInput").ap()
    locWT = nc.dram_tensor("locWT", [NF, ATT], F32, kind="ExternalInput").ap()
    vT = nc.dram_tensor("vT", [ATT, 1], BF16, kind="ExternalInput").ap()
    pmT = nc.dram_tensor("pmT", [BP, ATT, T], BF16, kind="ExternalInput").ap()
    # mask in transposed-chunk layout: [b, p, c] = maskadd[b, c*128 + p]
    maskT = nc.dram_tensor("maskT", [BP, 128, NT], F32, kind="ExternalInput").ap()
    # memory pre-tiled: [b, h, p, c*512+d] = mem[b, h*1024 + c*128 + p, d]
    mem = nc.dram_tensor("mem", [BP, 2, 128, 8 * ENC], BF16,
                         kind="ExternalInput").ap()

    out_ctx = nc.dram_tensor("out_ctx", [BP, ENC], F32, kind="ExternalOutput").ap()
    out_w = nc.dram_tensor("out_w", [BP, T], F32, kind="ExternalOutput").ap()

    with tile.TileContext(nc) as tc, ExitStack() as ctx:
        const_pool = ctx.enter_context(tc.tile_pool(name="const", bufs=1))
        xs_pool = ctx.enter_context(tc.tile_pool(name="xs", bufs=4))
        pm_pool = ctx.enter_context(tc.tile_pool(name="pm", bufs=4))
        mk_pool = ctx.enter_context(tc.tile_pool(name="mk", bufs=3))
        s_pool = ctx.enter_context(tc.tile_pool(name="s", bufs=4))
        th_pool = ctx.enter_context(tc.tile_pool(name="th", bufs=4))
        wu_pool = ctx.enter_context(tc.tile_pool(name="wu", bufs=3))
        sc_pool = ctx.enter_context(tc.tile_pool(name="sc", bufs=4))
        ow_pool = ctx.enter_context(tc.tile_pool(name="ow", bufs=3))
        mem_pool = ctx.enter_context(tc.tile_pool(name="mem", bufs=12))
        o_pool = ctx.enter_context(tc.tile_pool(name="o", bufs=4))
        ps_loc_pool = ctx.enter_context(tc.tile_pool(name="psloc", bufs=3, space="PSUM"))
        ps_e_pool = ctx.enter_context(tc.tile_pool(name="pse", bufs=2, space="PSUM"))
        ps_mc_pool = ctx.enter_context(tc.tile_pool(name="psmc", bufs=2, space="PSUM"))
        ps_ow_pool = ctx.enter_context(tc.tile_pool(name="psow", bufs=1, space="PSUM"))

        # ---- constants into SBUF ----
        qwt_t = const_pool.tile([128, (RNN // 128) * ATT], F32)
        ht_t = const_pool.tile([128, (RNN // 128) * BP], F32)
        for c in range(RNN // 128):
            nc.sync.dma_start(qwt_t[:, ts(c, ATT)], qWT[c * 128:(c + 1) * 128, :])
            nc.sync.dma_start(ht_t[:, ts(c, BP)], hiddenT[c * 128:(c + 1) * 128, :])
        cwr_t = const_pool.tile([NF, CK], F32)
        nc.sync.dma_start(cwr_t[:], cwr)
        locwt_t = const_pool.tile([NF, ATT], F32)
        nc.sync.dma_start(locwt_t[:], locWT)
        vt_t = const_pool.tile([ATT, 1], BF16)
        nc.sync.dma_start(vt_t[:], vT)
        ident_f = const_pool.tile([128, 128], F32)
        masks.make_identity(nc, ident_f[:])

        # ---- pq = query_W @ hidden.T : (ATT=128, BP=16), f32 ----
        ps_pq = ps_mc_pool.tile([ATT, BP], F32, tag="mc")
        for c in range(RNN // 128):
            nc.tensor.matmul(ps_pq[:], qwt_t[:, ts(c, ATT)], ht_t[:, ts(c, BP)],
                             start=(c == 0), stop=(c == RNN // 128 - 1))
        pq_t = const_pool.tile([ATT, BP], F32)
        nc.vector.tensor_copy(pq_t[:], ps_pq[:])

        # ---- CW2T[(c,k), a] = sum_f conv_W[f,(c,k)] loc_W[a,f] : (62, 128) ----
        ps_cw = ps_mc_pool.tile([CK, ATT], F32, tag="mc")
        nc.tensor.matmul(ps_cw[:], cwr_t[:], locwt_t[:], start=True, stop=True)
        cw2t_t = const_pool.tile([CK, ATT], BF16)
        nc.vector.tensor_copy(cw2t_t[:], ps_cw[:])

        mem_engs = [nc.sync, nc.gpsimd]
        wub_tiles = {}
        rinv_tiles = {}

        def ctx_part(b):
            # ---- context: ctx[b] = (sum_t wu[t] mem[t, :]) * rinv ----
            wub_t = wub_tiles.pop(b)
            rinv128_t = rinv_tiles.pop(b)
            ps_c = ps_mc_pool.tile([1, ENC], F32, tag="mc")
            for h in range(2):
                mem_t = mem_pool.tile([128, 8 * ENC], BF16)
                mem_engs[(2 * b + h) % 2].dma_start(mem_t[:], mem[b, h])
                for c in range(8):
                    si = h * 8 + c
                    nc.tensor.matmul(ps_c[:], wub_t[:, si:si + 1],
                                     mem_t[:, ts(c, ENC)],
                                     start=(si == 0), stop=(si == NT - 1))
            ctx_t = o_pool.tile([1, ENC], F32)
            nc.vector.tensor_scalar_mul(ctx_t[:], ps_c[:], rinv128_t[0:1, :])
            nc.scalar.dma_start(out_ctx[b:b + 1, :], ctx_t[:])

        for b in range(BP):
            # ---- energies (transposed): ps_eT[p, c] = e[b, c*128+p] ----
            xs_t = xs_pool.tile([CK, T], BF16)
            nc.gpsimd.dma_start(xs_t[:], xs[b])
            pm_t = pm_pool.tile([ATT, T], BF16)
            nc.gpsimd.dma_start(pm_t[:], pmT[b])
            mk_t = mk_pool.tile([128, NT], F32)
            nc.sync.dma_start(mk_t[:], maskT[b])

            ps_eT = ps_e_pool.tile([128, NT], F32)
            for c in range(4):
                ps_loc = ps_loc_pool.tile([ATT, 512], F32)
                nc.tensor.matmul(ps_loc[:], cw2t_t[:], xs_t[:, ts(c, 512)],
                                 start=True, stop=True)
                s_t = s_pool.tile([ATT, 512], F32)
                nc.vector.tensor_add(s_t[:], ps_loc[:], pm_t[:, ts(c, 512)])
                th_t = th_pool.tile([ATT, 512], BF16)
                nc.scalar.activation(th_t[:], s_t[:],
                                     mybir.ActivationFunctionType.Tanh,
                                     bias=pq_t[:, b:b + 1])
                for sl in range(4):
                    si = c * 4 + sl
                    nc.tensor.matmul(ps_eT[:, si:si + 1],
                                     th_t[:, ts(sl, 128)], vt_t[:],
                                     start=True, stop=True)

            # ---- masked exp (no max shift; |e| <= ||v||_1), sums ----
            emT_t = wu_pool.tile([128, NT], F32, tag="em")
            nc.vector.tensor_add(emT_t[:], ps_eT[:], mk_t[:])
            wuf_t = wu_pool.tile([128, NT], F32, tag="wuf")
            acc_t = sc_pool.tile([128, 1], F32, tag="acc")
            nc.scalar.activation(wuf_t[:], emT_t[:],
                                 mybir.ActivationFunctionType.Exp,
                                 accum_out=acc_t[:])
            wub_t = wu_pool.tile([128, NT], BF16, tag="wub")
            nc.vector.tensor_copy(wub_t[:], wuf_t[:])
            sumb_t = sc_pool.tile([128, 1], F32, tag="sumb")
            nc.gpsimd.partition_all_reduce(sumb_t[:], acc_t[:], channels=128,
                                           reduce_op=bass_isa.ReduceOp.add)
            rinv128_t = sc_pool.tile([128, 1], F32, tag="r128")
            nc.vector.reciprocal(rinv128_t[:], sumb_t[:])

            # ---- normalized weights row -> out_w[b] ----
            wnT_t = wu_pool.tile([128, NT], F32, tag="wn")
            nc.vector.tensor_scalar_mul(wnT_t[:], wuf_t[:], rinv128_t[:])
            ps_ow = ps_ow_pool.tile([NT, 128], F32)
            nc.tensor.transpose(ps_ow[:], wnT_t[:], ident_f[:])
            ow_t = ow_pool.tile([NT, 128], F32)
            nc.vector.tensor_copy(ow_t[:], ps_ow[:])
            nc.sync.dma_start(out_w[b:b + 1, :], ow_t[:])

            wub_tiles[b] = wub_t
            rinv_tiles[b] = rinv128_t
            if b >= 1:
                ctx_part(b - 1)
        ctx_part(BP - 1)

    nc.compile()
    return nc


def _marshal(inputs):
    """Full inputs -> per-core in_maps (host-side layout/dtype only)."""
    from ml_dtypes import bfloat16

    hid = np.ascontiguousarray(np.asarray(inputs["attention_hidden_state"], np.float32))
    memory = np.asarray(inputs["memory"], np.float32)
    pm = np.asarray(inputs["processed_memory"], np.float32)
    awc = np.asarray(inputs["attention_weights_cat"], np.float32)
    mask = np.asarray(inputs["mask"])
    qW = np.asarray(inputs["query_W"], np.float32)
    cW = np.asarray(inputs["conv_W"], np.float32)
    lW = np.asarray(inputs["loc_W"], np.float32)
    vW = np.asarray(inputs["v_W"], np.float32)

    hT = np.ascontiguousarray(hid.T)                       # (RNN, B)
    qWT = np.ascontiguousarray(qW.T)                       # (RNN, ATT)
    cwr = np.ascontiguousarray(cW.reshape(NF, CK))         # (32, 62)
    locWT = np.ascontiguousarray(lW.T)                     # (32, 128)
    vT = np.ascontiguousarray(vW.T).astype(bfloat16)       # (128, 1)

    xp = np.zeros((B, 2, T + 2 * PAD), np.float32)
    xp[:, :, PAD:PAD + T] = awc
    s0, s1, s2 = xp.strides
    xs_view = np.lib.stride_tricks.as_strided(
        xp, shape=(B, 2, KS, T), strides=(s0, s1, s2, s2))
    xs = np.ascontiguousarray(xs_view.reshape(B, CK, T)).astype(bfloat16)

    pmT = np.ascontiguousarray(pm.transpose(0, 2, 1)).astype(bfloat16)
    maskadd = np.where(mask, np.float32(-1e30), np.float32(0.0)).astype(np.float32)
    maskT = np.ascontiguousarray(
        maskadd.reshape(B, NT, 128).transpose(0, 2, 1))    # (B, 128, NT)
    # pre-tile memory: [b, h, p, c*512+d] = mem[b, h*1024 + c*128 + p, d]
    mem_bf = memory.astype(bfloat16)
    mem_tiled = np.ascontiguousarray(
        mem_bf.reshape(B, 2, 8, 128, ENC).transpose(0, 1, 3, 2, 4)
        .reshape(B, 2, 128, 8 * ENC))

    in_maps = []
    for c in range(N_CORES):
        sl = slice(c * BP, (c + 1) * BP)
        in_maps.append({
            "hiddenT": np.ascontiguousarray(hT[:, sl]),
            "qWT": qWT,
            "xs": xs[sl],
            "cwr": cwr,
            "locWT": locWT,
            "vT": vT,
            "pmT": pmT[sl],
            "maskT": maskT[sl],
            "mem": mem_tiled[sl],
        })
    return in_maps


def kernel(**inputs):
    global _NC_CACHE, LAST_RESULT
    if _NC_CACHE is None:
        _NC_CACHE = _build_nc()
    nc = _NC_CACHE
    in_maps = _marshal(inputs)
    res = run_bass_kernel_spmd(nc, in_maps, core_ids=list(range(N_CORES)),
                               trace=_TRACE)
    LAST_RESULT = res
    ctx = np.concatenate([r["out_ctx"] for r in res.results], axis=0)
    w = np.concatenate([r["out_w"] for r in res.results], axis=0)
    return ctx, w
